# revision 26
# baseline (speedup 1.0000x reference)
"""HBV hydrology model on 8 Trainium2 NeuronCores (Bass/Tile).

Strategy (hardcoded for T=730, G=10000, 8 cores):
  - Cells (basins) are data-parallel: 1250 cells/core, padded to 1280 = 10
    chunks of 128 partitions; inside the time loop every instruction covers
    all 1250 cells as a [128, 10] strided view of [128, 10*730] SBUF streams.
  - Host: parameter transforms (sigmoid->bounds, derived constants, gamma UH
    taps) and transposing x_phy fields to [cells, T] so DMA is contiguous.
  - Device phase A (per chunk, big-FD vector ops): SNOW / RAIN / signed
    melt-refreeze potential u streams from P and T.
  - Device phase B: the exact sequential snow/soil/upper-zone recurrence,
    730 steps of ~21 DVE ops (tensor_tensor + scalar_tensor_tensor), ln/exp
    and relus on the scalar (ACT) engine, side-branches on gpsimd.  The snow
    bucket uses the exact identity  MW2 = min(relu(MW+u), W');
    ts = (1+CWH)*relu(MW2 - c*W'); MW = MW2-ts; W = W'-ts
    with c = CWH/(1+CWH), W = SNOWPACK+MELTWATER, u = melt-refreeze potential.
  - Device phase C (per chunk): lower zone SLZ is linear -> one mult-add
    tensor_tensor_scan; Q0/Q1 recomputed vectorized from the stored SUZ1
    stream; 15-tap gamma unit hydrograph as shifted scalar_tensor_tensor
    (axpy) ops; DMA out.
"""

import math
import os
import sys

import numpy as np

if "/opt/trn_rl_repo" not in sys.path:
    sys.path.insert(0, "/opt/trn_rl_repo")

NEARZERO = 1e-5
LENF = 15
PHY_BOUNDS = [
    ("parBETA", 1.0, 6.0), ("parFC", 50.0, 1000.0), ("parK0", 0.05, 0.9),
    ("parK1", 0.01, 0.5), ("parK2", 0.001, 0.2), ("parLP", 0.2, 1.0),
    ("parPERC", 0.0, 10.0), ("parUZL", 0.0, 100.0), ("parTT", -2.5, 2.5),
    ("parCFMAX", 0.5, 10.0), ("parCFR", 0.0, 0.1), ("parCWH", 0.0, 0.2),
]

N_CORES = 8
T = 730
G = 10000
GPC = G // N_CORES          # 1250 cells per core
NCH = 10                    # chunks of 128 cells per core
GPAD = NCH * 128            # 1280

# PAR matrix columns (per cell):
#  0 TT, 1 CFMAX, 2 invFC, 3 BETA, 4 FC, 5 PERCp, 6 UZL, 7 K0, 8 K1c(=1-K1),
#  9 a(=1-K2), 10 k2a(=K2/(1-K2)), 11 negInvLPFC(=-1/(LP*FC)),
#  12 K01c(=K0*(1-K1)), 13 K1, 14 cwh(=CWH/(1+CWH)), 15 CFRC(=CFR*CFMAX),
#  16 CWH1(=1+CWH), 17..31 w0..w14 (UH taps)
NPAR = 17 + LENF

LAST_EXEC_NS = None


# ----------------------------------------------------------------------------
# custom DVE ops
# ----------------------------------------------------------------------------

def _register_custom_ops():
    import concourse.dve_ops as dve_ops
    from concourse.dve_spec import Spec, Src0, Src1, C0, C1, Zero, One, relu, minn

    def reg(name, body, reference):
        for op in dve_ops.OPS:
            if op.name == name:
                return op
        op = dve_ops.DveOp(name, Spec(body=body, reference=reference),
                           subdim=False, uops_sha={})
        dve_ops.OPS.append(op)
        dve_ops.CUSTOM_DVE_SPECS[name] = op.spec
        dve_ops._SUB_OPCODE_FOR_NAME[name] = (
            dve_ops._CUSTOM_DVE_ROW_BASE + len(dve_ops.OPS) - 1)
        assert dve_ops._SUB_OPCODE_FOR_NAME[name] < 0x20
        for ver in ("v3", "v4"):
            try:
                op.compile(ver)
            except ValueError as e:
                import re
                m = re.search(r"\b([0-9a-f]{16,})\b", str(e))
                if not m:
                    raise
                op.uops_sha[ver] = m.group(1)
                op.compile(ver)
        return op

    ops = {}
    # SNOW = P * (Tm - TT < 0)
    ops["SNOWSEL"] = reg(
        "HBV_SNOWSEL",
        Src0 * ((Src1 - C0) < Zero),
        lambda in0, in1, s0, s1, imm2: in0 * (in1 - s0 < 0),
    )
    # u = CFMAX*relu(Tm-TT) - CFRC*relu(TT-Tm)   (C0=CFMAX, C1=CFRC, s... Src0=Tm-TT)
    ops["USNOW"] = reg(
        "HBV_USNOW",
        C0 * relu(Src0) - C1 * relu(Zero - Src0),
        lambda in0, in1, s0, s1, imm2:
            s0 * np.maximum(in0, 0.0) - s1 * np.maximum(-in0, 0.0),
    )
    # min(relu(a), b)
    ops["MINRELU"] = reg(
        "HBV_MINRELU",
        minn(relu(Src0), Src1),
        lambda in0, in1, s0, s1, imm2: np.minimum(np.maximum(in0, 0.0), in1),
    )
    # 1 + a*b   (B = 1 + PET*negInvLPFC)
    ops["B1M"] = reg(
        "HBV_B1M",
        One + Src0 * Src1,
        lambda in0, in1, s0, s1, imm2: 1.0 + in0 * in1,
    )
    # relu(a - b), both tensors
    ops["RELUSUB"] = reg(
        "HBV_RELUSUB",
        relu(Src0 - Src1),
        lambda in0, in1, s0, s1, imm2: np.maximum(in0 - in1, 0.0),
    )
    # Src0*C0 + Src1*C1
    ops["QSUM"] = reg(
        "HBV_QSUM",
        Src0 * C0 + Src1 * C1,
        lambda in0, in1, s0, s1, imm2: in0 * s0 + in1 * s1,
    )
    return ops


# ----------------------------------------------------------------------------
# device program
# ----------------------------------------------------------------------------

def _patch_tile_drain():
    """The walrus build in this image caps sync-wait commands on SP control
    instructions at 2; Tile's kernel-tail drain can carry more.  Spill the
    global-clock waits over a chain of in-order SP NOPs instead."""
    import concourse.tile as tile_mod
    if getattr(tile_mod.TileContext, "_hbv_drain_patched", False):
        return
    from concourse.vector_clock import ScopedClock

    def _drain_and_barrier(self, tick_clock, wait_clock):
        carrier = self.nc.sync.nop(nofuse=True)
        wait_clock.add_sem_waits(
            carrier.ins, ScopedClock({None: tick_clock.global_clock}))
        si = carrier.ins.sync_info
        waits = list(si.on_wait) if si and si.on_wait else []
        if len(waits) > 1:
            si.on_wait = waits[:1]
            for i in range(1, len(waits)):
                extra = self.nc.sync.nop(nofuse=True)
                esi = extra.ins.sync_info
                if esi is None:
                    from concourse import mybir as _mybir
                    extra.ins.sync_info = _mybir.SyncInfo(
                        on_wait=waits[i:i + 1], on_update=[])
                else:
                    esi.on_wait = waits[i:i + 1]
        self.nc.sync.drain()
        self.nc.all_engine_barrier()
        popped = self.nc._tile_sem_poison_stack.pop()
        assert popped is self._sem_poison
        self.nc.clear_and_free_semaphores(list(self.sems.allocated().values()))
        self.nc.all_engine_barrier()

    tile_mod.TileContext._drain_and_barrier = _drain_and_barrier
    tile_mod.TileContext._hbv_drain_patched = True


def _split_sync_waits(nc, limit_ctrl=1, limit_compute=1):
    """This image's walrus caps per-instruction sync-wait commands (1 for
    control-class, ~2 for compute).  Spill excess waits onto preceding
    same-engine NoOps (engines execute their queue in order, so a wait on a
    preceding NoOp is equivalent)."""
    from concourse import mybir
    from concourse.instruction_name_ordered_set import InstructionNameOrderedSet
    ctrl_types = (mybir.InstNoOp, mybir.InstDrain, mybir.InstHalt,
                  mybir.InstEventSemaphore, mybir.InstAllEngineBarrier)
    n = [0]

    def fresh_nop(engine, wait, debug):
        n[0] += 1
        return mybir.InstNoOp(
            name=f"I-waitspill-{n[0]}", opcode="NoOp", engine=engine,
            debug=debug, ins=[], outs=[],
            descendants=InstructionNameOrderedSet(),
            sync_info=mybir.SyncInfo(on_wait=[wait], on_update=[]))

    for f in nc.m.functions:
        for bb in f.blocks:
            out = []
            changed = False
            for ins in bb.instructions:
                si = ins.sync_info
                waits = list(si.on_wait) if si and si.on_wait else []
                lim = limit_ctrl if isinstance(ins, ctrl_types) else limit_compute
                if len(waits) > lim:
                    for w in waits[:-lim] if lim else waits:
                        out.append(fresh_nop(ins.engine, w, ins.debug))
                    si.on_wait = waits[-lim:] if lim else []
                    changed = True
                out.append(ins)
            if changed:
                bb.instructions = out


def _build_nc(t_len=T, nch=NCH, split_waits=True):
    import concourse.bass as bass
    import concourse.tile as tile
    from concourse import mybir

    _patch_tile_drain()

    f32 = mybir.dt.float32
    Alu = mybir.AluOpType
    Act = mybir.ActivationFunctionType

    gpad = nch * 128

    nc = bass.Bass()
    P_d = nc.declare_dram_parameter("P", [gpad, t_len], f32, isOutput=False)
    TM_d = nc.declare_dram_parameter("TM", [gpad, t_len], f32, isOutput=False)
    PE_d = nc.declare_dram_parameter("PE", [gpad, t_len], f32, isOutput=False)
    PAR_d = nc.declare_dram_parameter("PAR", [gpad, NPAR], f32, isOutput=False)
    OUT_d = nc.declare_dram_parameter("OUT", [gpad, t_len], f32, isOutput=True)

    from contextlib import ExitStack
    with tile.TileContext(nc) as tc, ExitStack() as ctx:
        # persistent stream tiles
        big = ctx.enter_context(tc.tile_pool(name="big", bufs=1))
        SNO_s = big.tile([128, nch * t_len], f32, tag="SNO_s")
        U_s = big.tile([128, nch * t_len], f32, tag="U_s")
        RAI_s = big.tile([128, nch * t_len], f32, tag="RAI_s")
        PET_s = big.tile([128, nch * t_len], f32, tag="PET_s")
        SZ1_s = big.tile([128, nch * t_len], f32, tag="SZ1_s")
        par_all = big.tile([128, nch * NPAR], f32, tag="par_all")
        state = ctx.enter_context(tc.tile_pool(name="state", bufs=1))
        SM = state.tile([128, nch], f32, tag="SM")
        SUZ = state.tile([128, nch], f32, tag="SUZ")
        WST = state.tile([128, nch], f32, tag="WST")   # W = SNOWPACK + MELTWATER
        MW = state.tile([128, nch], f32, tag="MW")

        pa = ctx.enter_context(tc.tile_pool(name="pa", bufs=1))
        pc = ctx.enter_context(tc.tile_pool(name="pc", bufs=1))

        # ------------------- phase A: per-chunk precompute -------------------
        for c in range(nch):
            par_c = par_all[:, c * NPAR:(c + 1) * NPAR]
            nc.sync.dma_start(par_c, PAR_d[c * 128:(c + 1) * 128, :])

            pet_c = PET_s[:, c * t_len:(c + 1) * t_len]
            nc.sync.dma_start(pet_c, PE_d[c * 128:(c + 1) * 128, :])

            p_t = pa.tile([128, t_len], f32, tag="p_t")
            nc.sync.dma_start(p_t[:], P_d[c * 128:(c + 1) * 128, :])
            tm_t = pa.tile([128, t_len], f32, tag="tm_t")
            nc.sync.dma_start(tm_t[:], TM_d[c * 128:(c + 1) * 128, :])

            # t1d = Tm - TT
            t1d = pa.tile([128, t_len], f32, tag="t1d")
            nc.vector.tensor_scalar(t1d[:], tm_t[:], par_c[:, 0:1], None,
                                    op0=Alu.subtract)
            # SNOW = P * (t1d < 0); RAIN = P - SNOW
            sno_c = SNO_s[:, c * t_len:(c + 1) * t_len]
            msk = pa.tile([128, t_len], f32, tag="msk")
            nc.vector.tensor_scalar(msk[:], t1d[:], 0.0, None, op0=Alu.is_lt)
            nc.vector.tensor_tensor(sno_c, msk[:], p_t[:], op=Alu.mult)
            nc.vector.tensor_tensor(RAI_s[:, c * t_len:(c + 1) * t_len],
                                    p_t[:], sno_c, op=Alu.subtract)
            # u = CFMAX*relu(t1d) + CFRC*min(t1d, 0)
            mpp = pa.tile([128, t_len], f32, tag="mpp")
            nc.vector.tensor_scalar(mpp[:], t1d[:], 0.0, par_c[:, 1:2],
                                    op0=Alu.max, op1=Alu.mult)
            rpn = pa.tile([128, t_len], f32, tag="rpn")
            nc.vector.tensor_scalar(rpn[:], t1d[:], 0.0, par_c[:, 15:16],
                                    op0=Alu.min, op1=Alu.mult)
            nc.vector.tensor_tensor(U_s[:, c * t_len:(c + 1) * t_len],
                                    mpp[:], rpn[:], op=Alu.add)

        # ------------------- phase B: sequential recurrence ------------------
        nc.vector.memset(SM[:], 0.001)
        nc.vector.memset(SUZ[:], 0.001)
        nc.vector.memset(WST[:], 0.002)
        nc.vector.memset(MW[:], 0.001)

        # [128, nch*t_len] viewed as [128, nch, t_len]; step t slice -> [128, nch]
        SNO_v = SNO_s[:].rearrange("p (c t) -> p c t", t=t_len)
        U_v = U_s[:].rearrange("p (c t) -> p c t", t=t_len)
        RAI_v = RAI_s[:].rearrange("p (c t) -> p c t", t=t_len)
        PET_v = PET_s[:].rearrange("p (c t) -> p c t", t=t_len)
        SZ1_v = SZ1_s[:].rearrange("p (c t) -> p c t", t=t_len)
        par_v = par_all[:].rearrange("p (c k) -> p c k", k=NPAR)
        invFC = par_v[:, :, 2]
        BETA = par_v[:, :, 3]
        FC = par_v[:, :, 4]
        PERCp = par_v[:, :, 5]
        UZL = par_v[:, :, 6]
        K0 = par_v[:, :, 7]
        K1c = par_v[:, :, 8]
        negILPFC = par_v[:, :, 11]
        CWHc = par_v[:, :, 14]
        CWH1 = par_v[:, :, 16]

        lp = ctx.enter_context(tc.tile_pool(name="lp", bufs=6))
        for t in range(t_len):
            # --- snow bucket (exact) ---
            # W' = W + SNOW; MW2 = min(relu(MW+u), W');
            # ts = (1+CWH)*relu(MW2 - c*W'); MW = MW2 - ts; W = W' - ts
            w1 = lp.tile([128, nch], f32, tag="w1")
            nc.gpsimd.tensor_tensor(w1[:], WST[:], SNO_v[:, :, t], op=Alu.add)
            xx = lp.tile([128, nch], f32, tag="xx")
            nc.gpsimd.tensor_tensor(xx[:], MW[:], U_v[:, :, t], op=Alu.add)
            mwp = lp.tile([128, nch], f32, tag="mwp")
            nc.vector.scalar_tensor_tensor(mwp[:], xx[:], 0.0, w1[:],
                                           op0=Alu.max, op1=Alu.min)
            cw = lp.tile([128, nch], f32, tag="cw")
            nc.gpsimd.tensor_tensor(cw[:], CWHc, w1[:], op=Alu.mult)
            ddt = lp.tile([128, nch], f32, tag="ddt")
            nc.vector.tensor_tensor(ddt[:], mwp[:], cw[:], op=Alu.subtract)
            ts = lp.tile([128, nch], f32, tag="ts")
            nc.vector.scalar_tensor_tensor(ts[:], ddt[:], 0.0, CWH1,
                                           op0=Alu.max, op1=Alu.mult)
            nc.vector.tensor_tensor(MW[:], mwp[:], ts[:], op=Alu.subtract)
            nc.vector.tensor_tensor(WST[:], w1[:], ts[:], op=Alu.subtract)
            rt = lp.tile([128, nch], f32, tag="rt")
            nc.vector.tensor_tensor(rt[:], RAI_v[:, :, t], ts[:], op=Alu.add)
            # --- soil ---
            u1 = lp.tile([128, nch], f32, tag="u1")
            nc.vector.tensor_tensor(u1[:], SM[:], invFC, op=Alu.mult)
            lg = lp.tile([128, nch], f32, tag="lg")
            nc.scalar.activation(lg[:], u1[:], Act.Ln)
            bl = lp.tile([128, nch], f32, tag="bl")
            nc.vector.tensor_tensor(bl[:], BETA, lg[:], op=Alu.mult)
            sw = lp.tile([128, nch], f32, tag="sw")
            nc.scalar.activation(sw[:], bl[:], Act.Exp)
            rech = lp.tile([128, nch], f32, tag="rech")
            nc.vector.tensor_tensor(rech[:], rt[:], sw[:], op=Alu.mult)
            sm1 = lp.tile([128, nch], f32, tag="sm1")
            nc.vector.tensor_tensor(sm1[:], SM[:], rt[:], op=Alu.add)
            sm2 = lp.tile([128, nch], f32, tag="sm2")
            nc.vector.tensor_tensor(sm2[:], sm1[:], rech[:], op=Alu.subtract)
            smc = lp.tile([128, nch], f32, tag="smc")
            nc.vector.tensor_tensor(smc[:], sm2[:], FC, op=Alu.min)
            ex = lp.tile([128, nch], f32, tag="ex")
            nc.gpsimd.tensor_tensor(ex[:], sm2[:], smc[:], op=Alu.subtract)
            g1 = lp.tile([128, nch], f32, tag="g1")
            nc.gpsimd.tensor_tensor(g1[:], PET_v[:, :, t], negILPFC, op=Alu.mult)
            f1 = lp.tile([128, nch], f32, tag="f1")
            nc.vector.scalar_tensor_tensor(f1[:], g1[:], 1.0, smc[:],
                                           op0=Alu.add, op1=Alu.mult)
            f2 = lp.tile([128, nch], f32, tag="f2")
            nc.gpsimd.tensor_tensor(f2[:], smc[:], PET_v[:, :, t], op=Alu.subtract)
            # SM' = max(max(f1, eps), f2)
            nc.vector.scalar_tensor_tensor(SM[:], f1[:], NEARZERO, f2[:],
                                           op0=Alu.max, op1=Alu.max)
            # --- upper zone ---
            inn = lp.tile([128, nch], f32, tag="inn")
            nc.vector.tensor_tensor(inn[:], rech[:], ex[:], op=Alu.add)
            suz1 = SZ1_v[:, :, t]
            nc.vector.tensor_tensor(suz1, SUZ[:], inn[:], op=Alu.add)
            d1s = lp.tile([128, nch], f32, tag="d1s")
            nc.vector.tensor_tensor(d1s[:], suz1, PERCp, op=Alu.subtract)
            s2 = lp.tile([128, nch], f32, tag="s2")
            nc.scalar.activation(s2[:], d1s[:], Act.Relu)
            d2s = lp.tile([128, nch], f32, tag="d2s")
            nc.vector.tensor_tensor(d2s[:], s2[:], UZL, op=Alu.subtract)
            v0 = lp.tile([128, nch], f32, tag="v0")
            nc.vector.scalar_tensor_tensor(v0[:], d2s[:], 0.0, K0,
                                           op0=Alu.max, op1=Alu.mult)
            w2 = lp.tile([128, nch], f32, tag="w2")
            nc.vector.tensor_tensor(w2[:], s2[:], v0[:], op=Alu.subtract)
            nc.vector.tensor_tensor(SUZ[:], w2[:], K1c, op=Alu.mult)

        # ------------------- phase C: per-chunk postprocess ------------------
        for c in range(nch):
            par_c = par_all[:, c * NPAR:(c + 1) * NPAR]
            sz1_c = SZ1_s[:, c * t_len:(c + 1) * t_len]
            s2c = pc.tile([128, t_len], f32, tag="s2c")
            nc.vector.tensor_scalar(s2c[:], sz1_c, par_c[:, 5:6], 0.0,
                                    op0=Alu.subtract, op1=Alu.max)
            prc = pc.tile([128, t_len], f32, tag="prc")
            nc.gpsimd.tensor_tensor(prc[:], sz1_c, s2c[:], op=Alu.subtract)
            t1b = pc.tile([128, t_len], f32, tag="t1b")
            nc.vector.tensor_scalar(t1b[:], s2c[:], par_c[:, 6:7], 0.0,
                                    op0=Alu.subtract, op1=Alu.max)
            q1a = pc.tile([128, t_len], f32, tag="q1a")
            nc.vector.tensor_scalar_mul(q1a[:], s2c[:], par_c[:, 13:14])
            q01 = pc.tile([128, t_len], f32, tag="q01")
            nc.vector.scalar_tensor_tensor(q01[:], t1b[:], par_c[:, 12:13],
                                           q1a[:], op0=Alu.mult, op1=Alu.add)
            # SLZ scan: SLZ_t = a*SLZ_{t-1} + a*PRC_t
            a_bc = pc.tile([128, t_len], f32, tag="a_bc")
            nc.vector.tensor_scalar(a_bc[:], prc[:], 0.0, par_c[:, 9:10],
                                    op0=Alu.mult, op1=Alu.add)  # a_bc = a (bcast)
            prca = pc.tile([128, t_len], f32, tag="prca")
            nc.vector.tensor_scalar_mul(prca[:], prc[:], par_c[:, 9:10])
            slz = pc.tile([128, t_len], f32, tag="slz")
            nc.vector.tensor_tensor_scan(slz[:], a_bc[:], prca[:], 0.001,
                                         op0=Alu.mult, op1=Alu.add)
            qsim = pc.tile([128, t_len], f32, tag="qsim")
            nc.vector.scalar_tensor_tensor(qsim[:], slz[:], par_c[:, 10:11], q01[:],
                                           op0=Alu.mult, op1=Alu.add)
            # UH conv: y_t = sum_k w_k * q_{t-k}
            y = pc.tile([128, t_len], f32, tag="y")
            nc.vector.tensor_scalar_mul(y[:], qsim[:], par_c[:, 17:18])
            for k in range(1, LENF):
                nc.vector.scalar_tensor_tensor(
                    y[:, k:t_len], qsim[:, 0:t_len - k], par_c[:, 17 + k:18 + k],
                    y[:, k:t_len], op0=Alu.mult, op1=Alu.add)
            nc.sync.dma_start(OUT_d[c * 128:(c + 1) * 128, :], y[:])

    if split_waits:
        _split_sync_waits(nc)
    return nc


# ----------------------------------------------------------------------------
# host side
# ----------------------------------------------------------------------------

def _sigmoid(x):
    return 1.0 / (1.0 + np.exp(-x))


def _host_params(par_last):
    """par_last: [G, 14] raw -> PAR matrix [G, NPAR] float32."""
    g = par_last.shape[0]
    phy = _sigmoid(par_last[:, :12].astype(np.float64))
    p = {name: lo + phy[:, i] * (hi - lo)
         for i, (name, lo, hi) in enumerate(PHY_BOUNDS)}
    rout = _sigmoid(par_last[:, 12:].astype(np.float64))
    rout_a = rout[:, 0] * 2.9
    rout_b = rout[:, 1] * 6.5

    K1 = p['parK1']
    K2 = p['parK2']
    a = 1.0 - K2
    par = np.empty((g, NPAR), dtype=np.float64)
    par[:, 0] = p['parTT']
    par[:, 1] = p['parCFMAX']
    par[:, 2] = 1.0 / p['parFC']
    par[:, 3] = p['parBETA']
    par[:, 4] = p['parFC']
    par[:, 5] = p['parPERC']
    par[:, 6] = p['parUZL']
    par[:, 7] = p['parK0']
    par[:, 8] = 1.0 - K1
    par[:, 9] = a
    par[:, 10] = K2 / a
    par[:, 11] = -1.0 / (p['parLP'] * p['parFC'])
    par[:, 12] = p['parK0'] * (1.0 - K1)
    par[:, 13] = K1
    par[:, 14] = p['parCWH'] / (1.0 + p['parCWH'])
    par[:, 15] = p['parCFR'] * p['parCFMAX']
    par[:, 16] = 1.0 + p['parCWH']

    aa = np.maximum(rout_a, 0.0) + 0.1
    th = np.maximum(rout_b, 0.0) + 0.5
    tt = (np.arange(LENF, dtype=np.float64) + 0.5)[:, None]
    lg = np.array([math.lgamma(v) for v in aa])
    denom = np.exp(lg) * th ** aa
    w = tt ** (aa - 1.0) * np.exp(-tt / th) / denom
    w = w / np.sum(w, axis=0, keepdims=True)
    par[:, 17:17 + LENF] = w.T
    return par.astype(np.float32)


_NC_CACHE = {}


def _ensure_ntff_hook():
    """Install the axon NTFF profiling hook if the image's boot didn't."""
    try:
        import types

        try:
            import antenv.axon_hooks as ah
        except ImportError:
            import antenv
            ah = types.ModuleType("antenv.axon_hooks")
            ah._HOOK = None

            def _set(h, _m=ah):
                _m._HOOK = h

            def _get(_m=ah):
                return _m._HOOK

            ah.set_axon_ntff_profile_hook = _set
            ah.get_axon_ntff_profile_hook = _get
            sys.modules["antenv.axon_hooks"] = ah
            antenv.axon_hooks = ah
        if ah.get_axon_ntff_profile_hook() is None:
            from trn_agent_boot.trn_boot import _ntff_profile_via_ctypes
            hook = _ntff_profile_via_ctypes("/opt/axon/libaxon_pjrt.so")
            if hook is not None:
                ah.set_axon_ntff_profile_hook(hook)
        # artifact upload needs fishnet creds; degrade to a no-op
        import concourse.bass_utils as bu
        _orig_upload = bu.upload_artifacts

        def _safe_upload(tmpdir):
            try:
                return _orig_upload(tmpdir)
            except Exception:
                return tmpdir

        bu.upload_artifacts = _safe_upload
    except Exception:
        pass


def kernel(x_phy: np.ndarray, parameters: np.ndarray) -> np.ndarray:
    global LAST_EXEC_NS
    from concourse.bass_utils import run_bass_kernel_spmd

    x = np.asarray(x_phy, dtype=np.float32)
    par_full = _host_params(np.asarray(parameters[-1], dtype=np.float32))

    if "nc" not in _NC_CACHE:
        _NC_CACHE["nc"] = _build_nc()
    nc = _NC_CACHE["nc"]

    # benign padding params
    pad_par = np.zeros((1, NPAR), dtype=np.float32)
    pad_par[0, :17] = [0.0, 1.0, 0.01, 2.0, 100.0, 1.0, 10.0, 0.1, 0.9, 0.9,
                       0.111, -0.02, 0.09, 0.1, 0.1, 0.05, 1.1]
    pad_par[0, 17:] = 1.0 / LENF

    in_maps = []
    for i in range(N_CORES):
        lo, hi = i * GPC, (i + 1) * GPC
        m = {}
        for j, name in enumerate(("P", "TM", "PE")):
            arr = np.zeros((GPAD, T), dtype=np.float32)
            arr[:GPC] = np.ascontiguousarray(x[:, lo:hi, j].T)
            m[name] = arr
        pr = np.repeat(pad_par, GPAD, axis=0)
        pr[:GPC] = par_full[lo:hi]
        m["PAR"] = pr
        in_maps.append(m)

    trace = os.environ.get("HBV_TRACE", "1") == "1"
    if trace:
        _ensure_ntff_hook()
    tmpdir = os.environ.get("HBV_TRACE_DIR") or None
    try:
        res = run_bass_kernel_spmd(nc, in_maps, list(range(N_CORES)),
                                   trace=trace, tmpdir=tmpdir)
    except Exception:
        if not trace:
            raise
        res = run_bass_kernel_spmd(nc, in_maps, list(range(N_CORES)),
                                   trace=False)
    LAST_EXEC_NS = res.exec_time_ns

    out = np.empty((T, G), dtype=np.float32)
    for i in range(N_CORES):
        lo, hi = i * GPC, (i + 1) * GPC
        out[:, lo:hi] = np.asarray(res.results[i]["OUT"])[:GPC].T
    return out
